# revision 3
# baseline (speedup 1.0000x reference)
"""DifferentialAttention Trainium2 kernel, 8-core SPMD (tensor-parallel over heads).

Reference computation (B=2, T=2048, d_model=1024, H=16, D=64):
    qkv = x @ W_qkv                     -> q1,q2,k1,k2,v per head
    s_i = q_i k_i^T / sqrt(D) + causal_mask ; a_i = softmax(s_i)
    attn = a1 - clip(lam,0,1) * a2
    out  = (attn @ v) @ W_out

Sharding: 2 heads per core (16 heads / 8 cores). Each core computes the full
qkv projection for its heads' columns, the per-head attention, and a partial
out-projection (its 128 rows of W_out); host sums the 8 partial outputs.

Layout strategy (per core):
  - x is staged host-side transposed: xt[d_model, B*T], so the qkv projection
    runs as qkvT[c, i] = W_qkv_loc[:, c].T-contraction with xt as the moving
    operand, producing q/k in the transposed [head_dim, token] layout that the
    attention score matmuls consume directly (no on-device transposes of x/q/k).
  - Scores are computed transposed, sT[j, i] = (k_tile)^T-contract-q, so
    exp(sT) feeds the attn@v matmul as the moving operand with v (transposed
    on-device via the PE) as the stationary operand -- no p transposes.
  - The softmax normalizer is FUSED into the attn@v matmul: the stationary is
    [v_h | ones*64] (M=128), so PSUM rows 0-63 accumulate the numerator o and
    rows 64-127 accumulate 64 replicated copies of Z = sum_j exp(s[j,i]) for
    free (matmul cost is per moving column, independent of M).  This removes
    the separate ones-matmul Z pass entirely (-22% PE work).
  - Causality is exploited structurally: fully-masked j-tiles are skipped, and
    the diagonal 128-block is masked MULTIPLICATIVELY on the bf16 exp output
    (0/1 triangle, DVE 2x mode) instead of a -1e9 add on f32 PSUM scores.
  - The qkv projection and score matmuls run in bf16 (f32 PSUM accumulation),
    the final out-projection in float32r (full-rate fp32).
"""

import numpy as np

B, T, DM, H, D = 2, 2048, 1024, 16, 64
NCORES = 8
HL = H // NCORES          # heads per core = 2
NI = B * T                # 4096 token rows
IB = 512                  # i-block (query) width
JT = 128                  # j-tile (key) width
NIB = T // IB             # 4 i-blocks per batch
KO = DM // 128            # 8 contraction chunks

# rb (1/Z) lives at PSUM/SBUF partitions 64-127; the combine multiplies it
# against o at partitions 0-63.  True  -> rely on DVE reading operands at
# different partition bases.  False -> DMA-shift rb down to partitions 0-63.
USE_CROSS = True

_cached = {}


def _modules():
    if "mods" in _cached:
        return _cached["mods"]
    import sys
    try:
        import concourse.bass  # noqa: F401
    except ImportError:
        sys.path.insert(0, "/opt/trn_rl_repo")
    import concourse.bass as bass
    import concourse.bacc as bacc
    import concourse.mybir as mybir
    import concourse.tile as tile
    from concourse.bass_utils import run_bass_kernel_spmd
    from concourse.masks import make_identity
    _cached["mods"] = (bacc, mybir, tile, run_bass_kernel_spmd, make_identity)
    return _cached["mods"]


def build_nc(reps=1):
    """Build the single-core Bass program (identical on all 8 cores).

    reps>1 repeats the whole computation back-to-back in one NEFF --
    used only for timing (per-rep delta cancels dispatch overhead)."""
    key = ("nc", reps)
    if key in _cached:
        return _cached[key]
    bacc, mybir, tile, _, make_identity = _modules()
    f32 = mybir.dt.float32
    f32r = mybir.dt.float32r
    bf16 = mybir.dt.bfloat16
    EXP = mybir.ActivationFunctionType.Exp
    MUL = mybir.AluOpType.mult
    SUB = mybir.AluOpType.subtract

    nc = bacc.Bacc(None, target_bir_lowering=False, debug=False)

    xt = nc.dram_tensor("xt", [DM, NI], f32, kind="ExternalInput")
    wq = nc.dram_tensor("wq", [DM, 5 * 128], f32, kind="ExternalInput")
    wo = nc.dram_tensor("wo", [128, DM], f32, kind="ExternalInput")
    tri8 = nc.dram_tensor("tri8", [128, 128], f32, kind="ExternalInput")
    lam2 = nc.dram_tensor("lam2", [128, 2], f32, kind="ExternalInput")
    y = nc.dram_tensor("y", [NI, DM], f32, kind="ExternalOutput")

    xt_r = xt.rearrange("(ko ki) n -> ki ko n", ki=128)
    wq_r = wq.rearrange("(ko ki) c -> ki ko c", ki=128)

    with tile.TileContext(nc) as tc:
        with (
            tc.tile_pool(name="const", bufs=1) as const,
            tc.tile_pool(name="xts", bufs=2) as xts_p,
            tc.tile_pool(name="vtmp", bufs=2) as vtmp_p,
            tc.tile_pool(name="pp", bufs=4) as pp_p,
            tc.tile_pool(name="rb", bufs=4) as rb_p,
            tc.tile_pool(name="of", bufs=2) as of_p,
            tc.tile_pool(name="tmp", bufs=4) as tmp_p,
            tc.tile_pool(name="ys", bufs=3) as ys_p,
            tc.tile_pool(name="psw", bufs=3, space="PSUM") as psw,
            tc.tile_pool(name="psz", bufs=4, space="PSUM") as psz,
            tc.tile_pool(name="psq", bufs=1, space="PSUM") as psq,
        ):
            # ---- static tiles
            WQ = const.tile([128, KO, 5 * 128], bf16)
            nc.gpsimd.dma_start(WQ[:], wq_r)
            WO = const.tile([128, DM], f32r)
            nc.sync.dma_start(WO[:], wo[:].bitcast(f32r))
            TRI8 = const.tile([128, 128], bf16)
            nc.gpsimd.dma_start(TRI8[:], tri8[:])
            LAM2 = const.tile([128, 2], f32)
            nc.sync.dma_start(LAM2[:], lam2[:])
            IDN = const.tile([128, 128], f32)
            make_identity(nc, IDN[:])
            # persistent activations: q1/q2/k1/k2 chunks (c=0..3) and
            # v-natural extended with a ones block per head:
            #   VN2[j, jtile, h, 0:64]   = v_h^T
            #   VN2[j, jtile, h, 64:128] = 1.0   (Z-fusion columns)
            QKS = const.tile([128, 4, NI], bf16)      # [dim(A|B), chunk, token]
            VN2 = const.tile([128, B * 16, 2, 128], bf16)
            nc.vector.memset(VN2[:, :, :, 64:128], 1.0)

            def make_qkv_emitters(b, ib):
                """Per-chunk emission closures for pair (b, ib), so the next
                i-block's projection interleaves into the current attention
                loop (keeps ACT fed while PE fills exp-wait gaps)."""
                i0 = b * T + ib * IB
                state = {}

                def dma():
                    xts = xts_p.tile([128, KO, IB], bf16)
                    nc.gpsimd.dma_start(xts[:], xt_r[:, :, i0:i0 + IB])
                    state["xts"] = xts

                ems = [dma]

                def chunk(c):
                    xts = state["xts"]
                    ps = psq.tile([128, IB], f32, tag="q")
                    pslice = ps[:, 0:IB]
                    for k in range(KO):
                        nc.tensor.matmul(
                            pslice,
                            WQ[:, k, c * 128:(c + 1) * 128],
                            xts[:, k, :],
                            start=(k == 0),
                            stop=(k == KO - 1),
                        )
                    if c < 4:
                        nc.vector.tensor_copy(QKS[:, c, i0:i0 + IB], pslice)
                    else:
                        vts = vtmp_p.tile([128, IB], f32)
                        nc.vector.tensor_copy(vts[:], pslice)
                        for jb in range(IB // 128):
                            pst = psq.tile([128, IB], f32, tag="q")
                            nc.tensor.transpose(
                                pst[:, 0:128],
                                vts[:, jb * 128:(jb + 1) * 128],
                                IDN[:],
                            )
                            t = b * 16 + ib * 4 + jb
                            for h in range(2):
                                nc.vector.tensor_copy(
                                    VN2[:, t, h, 0:64],
                                    pst[:, h * 64:(h + 1) * 64],
                                )

                for c in range(5):
                    ems.append(lambda c=c: chunk(c))
                return ems

            pairs = [(b, ib) for b in range(B) for ib in range(NIB)] * reps
            for em in make_qkv_emitters(*pairs[0]):
                em()
            for idx, (b, ib) in enumerate(pairs):
                    i0 = b * T + ib * IB
                    pend = (
                        make_qkv_emitters(*pairs[idx + 1])
                        if idx + 1 < len(pairs) else []
                    )
                    # ================= attention for this i-block =============
                    # o_z[p][h]: PSUM rows 0-63 = numerator, 64-127 = Z copies
                    o_z = [[None, None], [None, None]]
                    rb = [[None, None], [None, None]]
                    njt = 4 * ib + 4   # causal: j-tiles 0 .. 4*ib+3
                    stride = max(1, (4 * njt) // (len(pend) + 1))
                    g = 0
                    for p in range(2):          # score 1 / score 2
                        qc, kc = p, 2 + p       # chunk ids of qT and kT
                        for h in range(2):
                            oz = psz.tile([128, IB], f32, tag="acc",
                                          name=f"oz{p}{h}")
                            o_z[p][h] = oz
                            for jt in range(njt):
                                r = jt - 4 * ib   # >=0 on the diagonal band
                                lo = 128 * r if r > 0 else 0
                                jq = b * T + jt * 128
                                st = psw.tile([128, IB], f32, tag="w")
                                nc.tensor.matmul(
                                    st[:, lo:IB],
                                    QKS[h * 64:(h + 1) * 64, kc, jq:jq + 128],
                                    QKS[h * 64:(h + 1) * 64, qc,
                                        i0 + lo:i0 + IB],
                                    start=True,
                                    stop=True,
                                    tile_position=(h * 64, 0),
                                )
                                pp = pp_p.tile([128, IB], bf16, tag="pp")
                                nc.scalar.activation(
                                    pp[:, lo:IB], st[:, lo:IB], EXP,
                                    scale=0.125,
                                )
                                if r >= 0:
                                    # multiplicative 0/1 triangle on the
                                    # diagonal 128-block (replaces -1e9 add)
                                    nc.vector.tensor_tensor(
                                        pp[:, lo:lo + 128],
                                        pp[:, lo:lo + 128],
                                        TRI8[:],
                                        MUL,
                                    )
                                nc.tensor.matmul(
                                    oz[:, lo:IB],
                                    VN2[:, b * 16 + jt, h, :],
                                    pp[:, lo:IB],
                                    start=(jt == 0),
                                    stop=(jt == njt - 1),
                                    skip_group_check=True,
                                )
                                g += 1
                                if pend and g % stride == 0:
                                    pend.pop(0)()
                            # normalizer for this (p, h): 1/Z straight off
                            # PSUM rows 64-127 (64 replicated rows)
                            r_t = rb_p.tile([128, IB], f32, tag="rb",
                                            name=f"rb{p}{h}")
                            nc.vector.reciprocal(r_t[64:128, :],
                                                 oz[64:128, :])
                            if p == 1:
                                nc.vector.tensor_scalar(
                                    r_t[64:128, :], r_t[64:128, :],
                                    LAM2[64:128, h:h + 1], None, MUL,
                                )
                            rb[p][h] = r_t
                    while pend:
                        pend.pop(0)()

                    # ================= combine ================================
                    OF = of_p.tile([128, IB], f32r)
                    for h in range(2):
                        if USE_CROSS:
                            rb0 = rb[0][h][64:128, :]
                            rb1 = rb[1][h][64:128, :]
                        else:
                            s0 = rb_p.tile([128, IB], f32, tag="rbs")
                            s1 = rb_p.tile([128, IB], f32, tag="rbs")
                            nc.sync.dma_start(s0[0:64, :], rb[0][h][64:128, :])
                            nc.sync.dma_start(s1[0:64, :], rb[1][h][64:128, :])
                            rb0 = s0[0:64, :]
                            rb1 = s1[0:64, :]
                        t1 = tmp_p.tile([128, IB], f32, tag="t")
                        t2 = tmp_p.tile([128, IB], f32, tag="t")
                        nc.vector.tensor_tensor(
                            t1[0:64, :], o_z[0][h][0:64, :], rb0, MUL)
                        nc.vector.tensor_tensor(
                            t2[0:64, :], o_z[1][h][0:64, :], rb1, MUL)
                        nc.vector.tensor_tensor(
                            OF[h * 64:(h + 1) * 64, :],
                            t1[0:64, :], t2[0:64, :], SUB)

                    # ================= out-projection (partial) ===============
                    for ic in range(IB // 128):
                        ysb = ys_p.tile([128, 1024], f32)
                        for nt in range(DM // 512):
                            pso = psz.tile([128, IB], f32, tag="acc",
                                           name=f"pso{nt}")
                            nc.tensor.matmul(
                                pso[:],
                                OF[:, ic * 128:(ic + 1) * 128],
                                WO[:, nt * 512:(nt + 1) * 512],
                                start=True,
                                stop=True,
                            )
                            nc.vector.tensor_copy(
                                ysb[:, nt * 512:(nt + 1) * 512], pso[:]
                            )
                        nc.sync.dma_start(
                            y[i0 + ic * 128:i0 + (ic + 1) * 128, :], ysb[:]
                        )

    nc.compile()
    _cached[key] = nc
    return nc


def make_in_maps(x, mask, W_qkv, W_out, lam):
    x = np.asarray(x, dtype=np.float32)
    mask = np.asarray(mask, dtype=np.float32)
    W_qkv = np.asarray(W_qkv, dtype=np.float32)
    W_out = np.asarray(W_out, dtype=np.float32)
    lam = np.asarray(lam, dtype=np.float32)

    xt = np.ascontiguousarray(x.reshape(NI, DM).T)            # [1024, 4096]
    # 0/1 multiplicative causal triangle in [j, i] orientation (1 iff j<=i)
    tri8 = np.ascontiguousarray(
        (mask[0, 0, :128, :128].T == 0.0).astype(np.float32)
    )
    lam_c = np.clip(lam, 0.0, 1.0)
    Wr = W_qkv.reshape(DM, H, 5, D)
    Wo_r = W_out.reshape(H, D, DM)
    in_maps = []
    for c in range(NCORES):
        hA, hB = 2 * c, 2 * c + 1
        wq_loc = np.ascontiguousarray(
            np.concatenate(
                [Wr[:, [hA, hB], t, :].reshape(DM, 2 * D) for t in range(5)],
                axis=1,
            )
        )                                                      # [1024, 640]
        wo_loc = np.ascontiguousarray(Wo_r[[hA, hB]].reshape(2 * D, DM))  # [128,1024]
        lam2 = np.empty((128, 2), dtype=np.float32)
        lam2[:, 0] = lam_c[hA]
        lam2[:, 1] = lam_c[hB]
        in_maps.append(
            {
                "xt": xt,
                "wq": wq_loc,
                "wo": wo_loc,
                "tri8": tri8,
                "lam2": lam2,
            }
        )
    return in_maps


def kernel(x, mask, W_qkv, W_out, lam):
    _, _, _, run_bass_kernel_spmd, _ = _modules()
    nc = build_nc()
    in_maps = make_in_maps(x, mask, W_qkv, W_out, lam)
    res = run_bass_kernel_spmd(nc, in_maps, core_ids=list(range(NCORES)))
    parts = [res.results[c]["y"] for c in range(NCORES)]
    y = parts[0].astype(np.float64)
    for p in parts[1:]:
        y = y + p
    return y.astype(np.float32).reshape(B, T, DM)


# revision 7
# speedup vs baseline: 1.1712x; 1.1712x over previous
"""DifferentialAttention Trainium2 kernel, 8-core SPMD (tensor-parallel over heads).

Reference computation (B=2, T=2048, d_model=1024, H=16, D=64):
    qkv = x @ W_qkv                     -> q1,q2,k1,k2,v per head
    s_i = q_i k_i^T / sqrt(D) + causal_mask ; a_i = softmax(s_i)
    attn = a1 - clip(lam,0,1) * a2
    out  = (attn @ v) @ W_out

Sharding: 2 heads per core (16 heads / 8 cores). Each core computes the full
qkv projection for its heads' columns, the per-head attention, and a partial
out-projection (its 128 rows of W_out); host sums the 8 partial outputs.

Layout strategy (per core):
  - x is staged host-side transposed: xt[d_model, B*T]; the qkv projection
    produces q in the transposed [head_dim, token] layout the score matmuls
    consume as the moving operand, and k zero-padded per head (KPAD) so the
    score matmuls run as full 128x128-mode matmuls (no PE tiling-mode
    switches anywhere in the main stream -> no pipeline drains, PE p-state
    stays ramped).
  - Scores are computed transposed, sT[j, i] = kpad^T-contract-q, both
    paths of one head into a single [128, 2, 512] PSUM supertile so one
    activation instruction exponentiates both paths (halves ACT overhead).
  - The softmax normalizer is FUSED into the attn@v matmul: the stationary is
    [v_h | ones*64] (M=128), so PSUM rows 0-63 accumulate the numerator o and
    rows 64-127 accumulate 64 replicated copies of Z = sum_j exp(s[j,i]) for
    free (matmul cost is per moving column, independent of M).  This removes
    the separate ones-matmul Z pass entirely (-22% PE work).
  - Causality is exploited structurally: fully-masked j-tiles are skipped, and
    the diagonal 128-block is masked MULTIPLICATIVELY on the bf16 exp output
    (0/1 triangle, DVE 2x mode) instead of a -1e9 add on f32 PSUM scores.
  - PE occupancy: attn@v for j-tile jt is emitted after the scores of jt+1
    (the PE computes scores while ACT exponentiates), the next i-block's
    projection matmuls are injected one-per-step into the attention stream,
    and each i-block's out-projection is deferred into the next i-block's
    stream so the PE never waits on the DVE combine tail.
"""

import numpy as np

B, T, DM, H, D = 2, 2048, 1024, 16, 64
NCORES = 8
HL = H // NCORES          # heads per core = 2
NI = B * T                # 4096 token rows
IB = 512                  # i-block (query) width
JT = 128                  # j-tile (key) width
NIB = T // IB             # 4 i-blocks per batch
KO = DM // 128            # 8 contraction chunks

_cached = {}


def _modules():
    if "mods" in _cached:
        return _cached["mods"]
    import sys
    try:
        import concourse.bass  # noqa: F401
    except ImportError:
        sys.path.insert(0, "/opt/trn_rl_repo")
    import concourse.bass as bass
    import concourse.bacc as bacc
    import concourse.mybir as mybir
    import concourse.tile as tile
    from concourse.bass_utils import run_bass_kernel_spmd
    from concourse.masks import make_identity
    _cached["mods"] = (bacc, mybir, tile, run_bass_kernel_spmd, make_identity)
    return _cached["mods"]


def build_nc(reps=1):
    """Build the single-core Bass program (identical on all 8 cores).

    reps>1 repeats the whole computation back-to-back in one NEFF --
    used only for timing (per-rep delta cancels dispatch overhead)."""
    key = ("nc", reps)
    if key in _cached:
        return _cached[key]
    bacc, mybir, tile, _, make_identity = _modules()
    f32 = mybir.dt.float32
    f32r = mybir.dt.float32r
    bf16 = mybir.dt.bfloat16
    EXP = mybir.ActivationFunctionType.Exp
    MUL = mybir.AluOpType.mult
    SUB = mybir.AluOpType.subtract

    nc = bacc.Bacc(None, target_bir_lowering=False, debug=False)

    xt = nc.dram_tensor("xt", [DM, NI], f32, kind="ExternalInput")
    wq = nc.dram_tensor("wq", [DM, 5 * 128], f32, kind="ExternalInput")
    wo = nc.dram_tensor("wo", [128, DM], f32, kind="ExternalInput")
    tri8 = nc.dram_tensor("tri8", [128, 128], f32, kind="ExternalInput")
    lam2 = nc.dram_tensor("lam2", [128, 2], f32, kind="ExternalInput")
    y = nc.dram_tensor("y", [NI, DM], f32, kind="ExternalOutput")

    xt_r = xt.rearrange("(ko ki) n -> ki ko n", ki=128)
    wq_r = wq.rearrange("(ko ki) c -> ki ko c", ki=128)

    with tile.TileContext(nc) as tc:
        with (
            tc.tile_pool(name="const", bufs=1) as const,
            tc.tile_pool(name="xts", bufs=2) as xts_p,
            tc.tile_pool(name="vtmp", bufs=2) as vtmp_p,
            tc.tile_pool(name="pp", bufs=3) as pp_p,
            tc.tile_pool(name="rb", bufs=4) as rb_p,
            tc.tile_pool(name="of", bufs=2) as of_p,
            tc.tile_pool(name="tmp", bufs=4) as tmp_p,
            tc.tile_pool(name="ys", bufs=3) as ys_p,
            tc.tile_pool(name="psw", bufs=2, space="PSUM") as psw,
            tc.tile_pool(name="psz", bufs=2, space="PSUM") as psz,
            tc.tile_pool(name="pso", bufs=1, space="PSUM") as pso_p,
            tc.tile_pool(name="psq", bufs=1, space="PSUM") as psq,
        ):
            # ---- static tiles
            WQ = const.tile([128, KO, 5 * 128], bf16)
            nc.gpsimd.dma_start(WQ[:], wq_r)
            WO = const.tile([128, DM], f32r)
            nc.sync.dma_start(WO[:], wo[:].bitcast(f32r))
            TRI8 = const.tile([128, 128], bf16)
            nc.gpsimd.dma_start(TRI8[:], tri8[:])
            LAM2 = const.tile([128, 2], f32)
            nc.sync.dma_start(LAM2[:], lam2[:])
            IDN = const.tile([128, 128], f32)
            make_identity(nc, IDN[:])
            # persistent activations:
            #   QS[dim(A|B), path, token]  : q1/q2 transposed (moving operand)
            #   KPAD[path][head]           : k zero-padded to K=128 stationary
            #   VN2[j, jtile, h, 0:64|64:] : v_h^T | 1.0  (Z-fusion columns)
            QS = const.tile([128, 2, NI], bf16)
            KPAD = [[const.tile([128, NI], bf16, name=f"kp{p}{h}")
                     for h in range(2)] for p in range(2)]
            for p in range(2):
                for h in range(2):
                    nc.vector.memset(KPAD[p][h][:], 0.0)
            VN2 = const.tile([128, B * 16, 2, 128], bf16)
            nc.vector.memset(VN2[:, :, :, 64:128], 1.0)

            def make_qkv_emitters(b, ib):
                """Fine-grained emission closures for pair (b, ib): one PE
                matmul (or copy/DMA) each, so the attention loop can inject
                exactly one between consecutive j-tile steps (keeps the PE
                busy through exp waits without long bursts)."""
                i0 = b * T + ib * IB
                state = {}

                def dma():
                    xts = xts_p.tile([128, KO, IB], bf16)
                    nc.gpsimd.dma_start(xts[:], xt_r[:, :, i0:i0 + IB])
                    state["xts"] = xts

                ems = [dma]

                def mm(c, k):
                    if k == 0:
                        state["ps"] = psq.tile([128, IB], f32, tag="q", name="qkv_ps")
                    nc.tensor.matmul(
                        state["ps"][:, 0:IB],
                        WQ[:, k, c * 128:(c + 1) * 128],
                        state["xts"][:, k, :],
                        start=(k == 0),
                        stop=(k == KO - 1),
                    )

                def out_q(c):
                    nc.vector.tensor_copy(QS[:, c, i0:i0 + IB],
                                          state["ps"][:, 0:IB])

                def out_k(c):
                    for h in range(2):
                        nc.vector.tensor_copy(
                            KPAD[c - 2][h][h * 64:(h + 1) * 64, i0:i0 + IB],
                            state["ps"][h * 64:(h + 1) * 64, 0:IB],
                        )

                def out_v_copy():
                    vts = vtmp_p.tile([128, IB], f32)
                    nc.vector.tensor_copy(vts[:], state["ps"][:, 0:IB])
                    state["vts"] = vts

                def out_v_tr(jb):
                    pst = psq.tile([128, IB], f32, tag="q")
                    nc.tensor.transpose(
                        pst[:, 0:128],
                        state["vts"][:, jb * 128:(jb + 1) * 128],
                        IDN[:],
                    )
                    t = b * 16 + ib * 4 + jb
                    nc.vector.tensor_copy(
                        VN2[:, t, :, 0:64],
                        pst[:, 0:128].rearrange("j (h d) -> j h d", h=2),
                    )

                for c in range(5):
                    for k in range(KO):
                        ems.append(lambda c=c, k=k: mm(c, k))
                    if c < 2:
                        ems.append(lambda c=c: out_q(c))
                    elif c < 4:
                        ems.append(lambda c=c: out_k(c))
                    else:
                        ems.append(out_v_copy)
                        for jb in range(IB // 128):
                            ems.append(lambda jb=jb: out_v_tr(jb))
                return ems

            def make_outproj_emitters(i0, OF):
                """Deferred out-projection for the i-block at i0 (stationary
                OF): emitted into the NEXT i-block's attention stream so the
                PE never stalls on the DVE combine tail."""
                ems = []

                def step(ic):
                    ysb = ys_p.tile([128, 1024], f32)
                    for nt in range(DM // 512):
                        pso = pso_p.tile([128, IB], f32, tag="o",
                                         name=f"pso{nt}")
                        nc.tensor.matmul(
                            pso[:],
                            OF[:, ic * 128:(ic + 1) * 128],
                            WO[:, nt * 512:(nt + 1) * 512],
                            start=True,
                            stop=True,
                        )
                        nc.vector.tensor_copy(
                            ysb[:, nt * 512:(nt + 1) * 512], pso[:]
                        )
                    nc.sync.dma_start(
                        y[i0 + ic * 128:i0 + (ic + 1) * 128, :], ysb[:]
                    )

                for ic in range(IB // 128):
                    ems.append(lambda ic=ic: step(ic))
                return ems

            pairs = [(b, ib) for b in range(B) for ib in range(NIB)] * reps
            for em in make_qkv_emitters(*pairs[0]):
                em()
            pend_op = []    # deferred out-projection of the previous i-block
            for idx, (b, ib) in enumerate(pairs):
                    i0 = b * T + ib * IB
                    pend = list(pend_op)
                    pend_op = []
                    if idx + 1 < len(pairs):
                        pend += make_qkv_emitters(*pairs[idx + 1])
                    njt = 4 * ib + 4   # causal: j-tiles 0 .. 4*ib+3
                    nsteps = 2 * (njt + 1)
                    stride = max(1, nsteps // (len(pend) + 1))
                    g = 0
                    OF = of_p.tile([128, IB], f32r)
                    for h in range(2):
                        oz = [psz.tile([128, IB], f32, tag="acc",
                                       name=f"oz{p}{h}") for p in range(2)]
                        pp_prev = None

                        def av(jt, pp_t, start, stop, h=h, oz=oz, b=b, ib=ib):
                            r = jt - 4 * ib
                            lo = 128 * r if r > 0 else 0
                            for p in range(2):
                                nc.tensor.matmul(
                                    oz[p][:, lo:IB],
                                    VN2[:, b * 16 + jt, h, :],
                                    pp_t[:, p, lo:IB],
                                    start=start,
                                    stop=stop,
                                    skip_group_check=True,
                                )

                        for jt in range(njt):
                            r = jt - 4 * ib   # >=0 on the diagonal band
                            lo = 128 * r if r > 0 else 0
                            jq = b * T + jt * 128
                            st = psw.tile([128, 2, IB], f32, tag="w")
                            for p in range(2):
                                nc.tensor.matmul(
                                    st[:, p, lo:IB],
                                    KPAD[p][h][:, jq:jq + 128],
                                    QS[:, p, i0 + lo:i0 + IB],
                                    start=True,
                                    stop=True,
                                )
                            pp = pp_p.tile([128, 2, IB], bf16, tag="pp")
                            nc.scalar.activation(
                                pp[:, :, lo:IB], st[:, :, lo:IB], EXP,
                                scale=0.125,
                            )
                            if r >= 0:
                                # multiplicative 0/1 triangle on the
                                # diagonal 128-block (replaces -1e9 add)
                                nc.vector.tensor_tensor(
                                    pp[:, :, lo:lo + 128],
                                    pp[:, :, lo:lo + 128],
                                    TRI8[:, None, :].to_broadcast(
                                        (128, 2, 128)),
                                    MUL,
                                )
                            # attn@v lags one j-tile: PE computes scores of
                            # jt while ACT exponentiates jt-1
                            if pp_prev is not None:
                                av(jt - 1, pp_prev, jt == 1, False)
                            pp_prev = pp
                            g += 1
                            if pend and g % stride == 0:
                                pend.pop(0)()
                        av(njt - 1, pp_prev, njt == 1, True)
                        g += 1
                        if pend and g % stride == 0:
                            pend.pop(0)()

                        # normalizers + combine for this head
                        rb = [None, None]
                        for p in range(2):
                            r_t = rb_p.tile([128, IB], f32, tag="rb",
                                            name=f"rb{p}{h}")
                            nc.vector.reciprocal(r_t[64:128, :],
                                                 oz[p][64:128, :])
                            if p == 1:
                                nc.vector.tensor_scalar(
                                    r_t[64:128, :], r_t[64:128, :],
                                    LAM2[64:128, h:h + 1], None, MUL,
                                )
                            rb[p] = r_t
                        t1 = tmp_p.tile([128, IB], f32, tag="t")
                        t2 = tmp_p.tile([128, IB], f32, tag="t")
                        nc.vector.tensor_tensor(
                            t1[0:64, :], oz[0][0:64, :], rb[0][64:128, :], MUL)
                        nc.vector.tensor_tensor(
                            t2[0:64, :], oz[1][0:64, :], rb[1][64:128, :], MUL)
                        nc.vector.tensor_tensor(
                            OF[h * 64:(h + 1) * 64, :],
                            t1[0:64, :], t2[0:64, :], SUB)
                    while pend:
                        pend.pop(0)()

                    op_ems = make_outproj_emitters(i0, OF)
                    if idx + 1 < len(pairs):
                        pend_op = op_ems
                    else:
                        for em in op_ems:
                            em()

    nc.compile()
    _cached[key] = nc
    return nc


def make_in_maps(x, mask, W_qkv, W_out, lam):
    x = np.asarray(x, dtype=np.float32)
    mask = np.asarray(mask, dtype=np.float32)
    W_qkv = np.asarray(W_qkv, dtype=np.float32)
    W_out = np.asarray(W_out, dtype=np.float32)
    lam = np.asarray(lam, dtype=np.float32)

    xt = np.ascontiguousarray(x.reshape(NI, DM).T)            # [1024, 4096]
    # 0/1 multiplicative causal triangle in [j, i] orientation (1 iff j<=i)
    tri8 = np.ascontiguousarray(
        (mask[0, 0, :128, :128].T == 0.0).astype(np.float32)
    )
    lam_c = np.clip(lam, 0.0, 1.0)
    Wr = W_qkv.reshape(DM, H, 5, D)
    Wo_r = W_out.reshape(H, D, DM)
    in_maps = []
    for c in range(NCORES):
        hA, hB = 2 * c, 2 * c + 1
        wq_loc = np.ascontiguousarray(
            np.concatenate(
                [Wr[:, [hA, hB], t, :].reshape(DM, 2 * D) for t in range(5)],
                axis=1,
            )
        )                                                      # [1024, 640]
        wo_loc = np.ascontiguousarray(Wo_r[[hA, hB]].reshape(2 * D, DM))  # [128,1024]
        lam2 = np.empty((128, 2), dtype=np.float32)
        lam2[:, 0] = lam_c[hA]
        lam2[:, 1] = lam_c[hB]
        in_maps.append(
            {
                "xt": xt,
                "wq": wq_loc,
                "wo": wo_loc,
                "tri8": tri8,
                "lam2": lam2,
            }
        )
    return in_maps


def kernel(x, mask, W_qkv, W_out, lam):
    _, _, _, run_bass_kernel_spmd, _ = _modules()
    nc = build_nc()
    in_maps = make_in_maps(x, mask, W_qkv, W_out, lam)
    res = run_bass_kernel_spmd(nc, in_maps, core_ids=list(range(NCORES)))
    parts = [res.results[c]["y"] for c in range(NCORES)]
    y = parts[0].astype(np.float64)
    for p in parts[1:]:
        y = y + p
    return y.astype(np.float32).reshape(B, T, DM)
